# revision 7
# baseline (speedup 1.0000x reference)
"""Trainium2 Bass kernel for the Gaussian density calculator.

density[g] = sum_a mask_a * sum_n aw[e_a,n] * exp(bw[e_a,n] * ||g - X_a||^2)

Strategy (self-contained; hardcoded for 8 NeuronCores):
 - Host: drop masked atoms, spatially sort the grid into 128-point tiles
   (4x4x8 lattice boxes), and keep only (atom, gaussian) pairs whose peak
   contribution anywhere in the tile can exceed EPS:
       log(aw) + bw * d_min^2 >= log(EPS)
   (d_min = distance from atom to the tile bounding box).  This value-based
   cull includes aw -- pairs with small coefficients die much earlier than
   under a plain distance cutoff.
 - The exponent is affine in per-point features:
       arg = bw*|g'|^2 - 2bw*(g'.X') + (bw*|X'|^2 + log aw)
           = [ |g'|^2, g'x, g'y, g'z, 1 ] . w5[:, pair]
   with coordinates recentred per tile.  On the recentred lattice every
   tile shares ONE exact-bf16 feature block, so the stationary matmul
   operand is a single [15,128] table loaded once.
 - fp32-accurate matmul on the bf16 PE datapath: W is split into 3 bf16
   components stacked along the contraction dim (G is exact bf16), K=15.
 - Pair columns are packed densely: slot k (one tile per core) owns a
   contiguous run of pad_k columns (pad_k = max pair count over the 8
   cores, rounded to a multiple of 8).  The column axis is then simply
   sliced into 512-wide PSUM banks -- ONE matmul per bank, one exp
   ACTIVATE per 2-bank group, one segmented (width-8) TENSOR_REDUCE per
   group.  Per-core output is the [128, WTOT/8] matrix of segment sums;
   the host adds the few segments of each slot in fp64 and unsorts.
 - Instruction count per core is ~100 (vs ~1000 for per-item matmuls),
   so no engine is sequencer-issue bound.
"""
import numpy as np
import ml_dtypes

import concourse.bacc as bacc
import concourse.tile as tile
from concourse import mybir
from concourse.bass_utils import run_bass_kernel_spmd

P = 128
NCORES = 8
EXCLUDED_ELEM = 5
EPS = 3e-2              # min kept peak term value (density scale is ~46)
SEG = 8                 # reduce segment width; slot widths pad to this
BANK = 512              # fp32 cols per PSUM bank (= max matmul width)
GROUP_BANKS = 2         # banks per exp/reduce group
NEG_BIG = -1e30
KR = 15                 # contraction rows: 5 features x 3 W bands
BF16 = ml_dtypes.bfloat16


def _split3(x):
    a0 = x.astype(BF16)
    r1 = x - a0.astype(np.float64)
    a1 = r1.astype(BF16)
    r2 = r1 - a1.astype(np.float64)
    a2 = r2.astype(BF16)
    return a0, a1, a2


def _prepare(grid_points, X, aw_table, bw_table, elements, C_expand):
    gp = np.asarray(grid_points).astype(np.float64)
    Ng = gp.shape[0]

    mask = (np.asarray(elements) != EXCLUDED_ELEM) & (np.asarray(C_expand) == 1)
    Xa = np.asarray(X).astype(np.float64)[mask]
    el = np.asarray(elements)[mask]
    aw = np.asarray(aw_table).astype(np.float64)[el]
    bw = np.asarray(bw_table).astype(np.float64)[el]
    logaw = np.where(aw > 0, np.log(np.maximum(aw, 1e-300)), NEG_BIG)

    # ---- spatial sort into tiles of 128 points ----
    ntiles = -(-Ng // P)
    ntiles = -(-ntiles // NCORES) * NCORES
    cell = np.floor(gp / np.array([2.0, 2.0, 4.0]))
    order = np.lexsort((cell[:, 2], cell[:, 1], cell[:, 0]))
    npad = ntiles * P - Ng
    order_padded = np.concatenate([order, np.full(npad, order[-1], np.int64)])
    gp_s = gp[order_padded].reshape(ntiles, P, 3)

    lo = gp_s.min(axis=1)
    hi = gp_s.max(axis=1)
    center = (lo + hi) / 2

    # ---- per-tile (atom, gaussian) pair selection, value-based ----
    d = np.maximum(lo[:, None, :] - Xa[None], Xa[None] - hi[:, None, :])
    d2 = (np.maximum(d, 0.0) ** 2).sum(-1)
    incl = logaw[None] + bw[None] * d2[:, :, None] >= np.log(EPS)  # [T,Na,6]
    cnt = incl.reshape(ntiles, -1).sum(1)

    # ---- deal tiles to cores by workload rank ----
    nslots = ntiles // NCORES
    rank = np.argsort(-cnt, kind="stable")
    tilemap = rank.reshape(nslots, NCORES)                # [k, c] -> tile id
    pad_k = np.maximum(cnt[tilemap].max(1), SEG)
    pad_k = -(-pad_k // SEG) * SEG
    off = np.concatenate([[0], np.cumsum(pad_k)])         # slot col offsets
    wtot = int(off[-1])

    # ---- shared G pattern (recentred lattice is identical per tile) ----
    gprime = gp_s - center[:, None, :]
    g5 = np.empty((ntiles, 5, P))
    g5[:, 0] = (gprime ** 2).sum(-1)
    g5[:, 1:4] = np.swapaxes(gprime, 1, 2)
    g5[:, 4] = 1.0
    assert np.all(g5 == g5[0]), "tiles do not share one feature pattern"
    assert np.all(g5[0] == g5[0].astype(BF16).astype(np.float64)), \
        "pattern not exact in bf16"
    g0 = g5[0].astype(BF16)
    G = np.concatenate([g0, g0, g0], axis=0)              # [15, 128]

    # ---- per-core W [15, WTOT] via one vectorized 3-band split ----
    W5 = np.empty((NCORES, 5, wtot))
    W5[:] = np.array([0, 0, 0, 0, NEG_BIG])[None, :, None]
    pair_an = [np.nonzero(incl[t]) for t in range(ntiles)]
    for k in range(nslots):
        q0 = off[k]
        for c in range(NCORES):
            t = int(tilemap[k, c])
            a_i, n_i = pair_an[t]
            m = a_i.shape[0]
            if m == 0:
                continue
            Xp = Xa[a_i] - center[t]
            bwi = bw[a_i, n_i]
            W5[c, 0, q0:q0 + m] = bwi
            W5[c, 1:4, q0:q0 + m] = -2.0 * bwi * Xp.T
            W5[c, 4, q0:q0 + m] = bwi * (Xp ** 2).sum(-1) + logaw[a_i, n_i]
    w0, w1, w2 = _split3(W5)
    Wc = np.concatenate([w0, w1, w2], axis=1)             # [NC, 15, WTOT]

    # ---- banks / groups / DMA chunks ----
    nbanks = -(-wtot // BANK)
    banks = [(b * BANK, min(BANK, wtot - b * BANK)) for b in range(nbanks)]
    if nbanks > 5:
        cb = [1, 3, 5, nbanks]                            # chunk ends (banks)
    elif nbanks > 3:
        cb = [1, 3, nbanks]
    else:
        cb = [nbanks]
    chunks = []                                           # (col0, col1)
    prev = 0
    for e in cb:
        c1 = min(e * BANK, wtot)
        chunks.append((prev, c1))
        prev = c1
    # pair banks into groups; a small trailing bank gets its own group so
    # the final reduce + store cover only a sliver
    gidx = []
    b = 0
    while b < nbanks:
        if b + 1 < nbanks and not (b + 2 == nbanks and banks[-1][1] < 256):
            gidx.append([b, b + 1])
            b += 2
        else:
            gidx.append([b])
            b += 1
    groups = []
    for gb in gidx:
        gbanks = []
        gw = 0
        for bi in gb:
            col0, w = banks[bi]
            ci = next(i for i, (c0, c1) in enumerate(chunks)
                      if c0 <= col0 < c1)
            gbanks.append(dict(loc=gw, w=w, ci=ci, src0=col0 - chunks[ci][0]))
            gw += w
        groups.append(dict(banks=gbanks, gw=gw,
                           seg0=(banks[gb[0]][0]) // SEG))
    meta = dict(
        wtot=wtot, chunks=chunks, groups=groups, off=off, pad_k=pad_k,
        nslots=nslots, tilemap=tilemap, order_padded=order_padded,
        Ng=Ng, ntiles=ntiles,
    )
    return G, Wc, meta


def _build_program(meta):
    nc = bacc.Bacc("TRN2", target_bir_lowering=False, debug=False,
                   num_devices=NCORES)
    wtot, chunks, groups = meta["wtot"], meta["chunks"], meta["groups"]
    nseg = wtot // SEG
    g_d = nc.dram_tensor("gp", [KR, P], mybir.dt.bfloat16,
                         kind="ExternalInput")
    w_d = [nc.dram_tensor(f"w{i}", [KR, c1 - c0], mybir.dt.bfloat16,
                          kind="ExternalInput")
           for i, (c0, c1) in enumerate(chunks)]
    out_d = nc.dram_tensor("out", [P, nseg], mybir.dt.float16,
                           kind="ExternalOutput")

    with tile.TileContext(nc) as tc:
        with (
            tc.tile_pool(name="data", bufs=1) as data,
            tc.tile_pool(name="ps", bufs=3, space="PSUM") as ps,
            tc.tile_pool(name="wps", bufs=1, space="PSUM") as wps,
            tc.tile_pool(name="work", bufs=4) as work,
            tc.tile_pool(name="accp", bufs=4) as accp,
        ):
            g_sb = data.tile([KR, P], mybir.dt.bfloat16)
            nc.scalar.dma_start(g_sb[:], g_d[:])
            w_sb = []
            wq = [nc.sync, nc.gpsimd, nc.sync, nc.gpsimd]
            for i, (c0, c1) in enumerate(chunks):
                wt = data.tile([KR, c1 - c0], mybir.dt.bfloat16, tag=f"w{i}")
                wq[i % len(wq)].dma_start(wt[:], w_d[i][:])
                w_sb.append(wt)
            # dependency-free warm-up: pulls the exp ACT_TABLE_LOAD into
            # the preamble instead of stalling the first real group
            wu = work.tile([P, 2], mybir.dt.float32, tag="wu")
            nc.vector.memset(wu[:], 0.0)
            nc.scalar.activation(out=wu[:], in_=wu[:],
                                 func=mybir.ActivationFunctionType.Exp)
            # PE warm-up: dummy matmuls on zeroed operands bridge the input
            # DMA wait so the tensor clock is fully ramped (and the array
            # busy) the moment real operands land
            wz = data.tile([KR, BANK], mybir.dt.bfloat16, tag="wz")
            nc.vector.memset(wz[:], 0.0)
            wp = wps.tile([P, BANK], mybir.dt.float32)
            for _ in range(8):
                nc.tensor.matmul(wp[:, :], wz[:, :P], wz[:, :],
                                 start=True, stop=True)
            for grp in groups:
                gw, ns = grp["gw"], grp["gw"] // SEG
                pst = ps.tile([P, GROUP_BANKS * BANK], mybir.dt.float32,
                              tag="ps")
                for bk in grp["banks"]:
                    nc.tensor.matmul(
                        pst[:, bk["loc"]:bk["loc"] + bk["w"]],
                        g_sb[:, :],
                        w_sb[bk["ci"]][:, bk["src0"]:bk["src0"] + bk["w"]],
                        start=True, stop=True,
                    )
                e3 = work.tile([P, GROUP_BANKS * BANK // SEG, SEG],
                               mybir.dt.float16, tag="e")
                nc.scalar.activation(out=e3[:, :ns, :], in_=pst[:, :gw],
                                     func=mybir.ActivationFunctionType.Exp)
                ac = accp.tile([P, GROUP_BANKS * BANK // SEG],
                               mybir.dt.float16, tag="acc")
                with nc.allow_low_precision("bounded fp16 segment sums; "
                                            "host accumulates in fp64"):
                    nc.vector.tensor_reduce(
                        ac[:, :ns], e3[:, :ns, :],
                        axis=mybir.AxisListType.X, op=mybir.AluOpType.add,
                    )
                nc.sync.dma_start(out_d[:, grp["seg0"]:grp["seg0"] + ns],
                                  ac[:, :ns])
    nc.compile()
    return nc


def _assemble(res, meta):
    ntiles, Ng, nslots = meta["ntiles"], meta["Ng"], meta["nslots"]
    tilemap, off = meta["tilemap"], meta["off"]
    starts = (off[:-1] // SEG).astype(np.int64)
    dens_tiles = np.zeros((ntiles, P), np.float32)
    for c in range(NCORES):
        o = res.results[c]["out"].astype(np.float64)      # [P, nseg]
        red = np.add.reduceat(o, starts, axis=1)          # [P, nslots]
        dens_tiles[tilemap[:, c]] = red.T.astype(np.float32)
    dens = np.zeros(Ng, np.float32)
    dens[meta["order_padded"][:Ng]] = dens_tiles.reshape(-1)[:Ng]
    side = round(Ng ** (1 / 3))
    if side ** 3 == Ng:
        return dens.reshape(side, side, side)
    return dens


def _in_maps(G, Wc, meta):
    maps = []
    for c in range(NCORES):
        m = {"gp": np.ascontiguousarray(G)}
        for i, (c0, c1) in enumerate(meta["chunks"]):
            m[f"w{i}"] = np.ascontiguousarray(Wc[c, :, c0:c1])
        maps.append(m)
    return maps


def kernel(grid_points, X, aw_table, bw_table, elements, C_expand):
    G, Wc, meta = _prepare(grid_points, X, aw_table, bw_table,
                           elements, C_expand)
    nc = _build_program(meta)
    res = run_bass_kernel_spmd(nc, _in_maps(G, Wc, meta),
                               list(range(NCORES)))
    return _assemble(res, meta)


# revision 9
# speedup vs baseline: 1.0612x; 1.0612x over previous
"""Trainium2 Bass kernel for the Gaussian density calculator.

density[g] = sum_a mask_a * sum_n aw[e_a,n] * exp(bw[e_a,n] * ||g - X_a||^2)

Strategy (self-contained; hardcoded for 8 NeuronCores):
 - Host: drop masked atoms, spatially sort the grid into 128-point tiles
   (4x4x8 lattice boxes), and keep only (atom, gaussian) pairs whose peak
   contribution anywhere in the tile can exceed EPS:
       log(aw) + bw * d_min^2 >= log(EPS)
   (d_min = distance from atom to the tile bounding box).  This value-based
   cull includes aw -- pairs with small coefficients die much earlier than
   under a plain distance cutoff.
 - The exponent is affine in per-point features:
       arg = bw*|g'|^2 - 2bw*(g'.X') + (bw*|X'|^2 + log aw)
           = [ |g'|^2, g'x, g'y, g'z, 1 ] . w5[:, pair]
   with coordinates recentred per tile.  On the recentred lattice every
   tile shares ONE exact-bf16 feature block, so the stationary matmul
   operand is a single [15,128] table loaded once.
 - fp32-accurate matmul on the bf16 PE datapath: W is split into 3 bf16
   components stacked along the contraction dim (G is exact bf16), K=15.
 - Pair columns are packed densely: slot k (one tile per core) owns a
   contiguous run of pad_k columns (pad_k = max pair count over the 8
   cores, rounded to a multiple of 8).  The column axis is then simply
   sliced into 512-wide PSUM banks -- ONE matmul per bank, one exp
   ACTIVATE per 2-bank group, one segmented (width-8) TENSOR_REDUCE per
   group.  Per-core output is the [128, WTOT/8] matrix of segment sums;
   the host adds the few segments of each slot in fp64 and unsorts.
 - Instruction count per core is ~100 (vs ~1000 for per-item matmuls),
   so no engine is sequencer-issue bound.
"""
import numpy as np
import ml_dtypes

import concourse.bacc as bacc
import concourse.tile as tile
from concourse import mybir
from concourse.bass_utils import run_bass_kernel_spmd

P = 128
NCORES = 8
EXCLUDED_ELEM = 5
EPS = 3e-2              # min kept peak term value (density scale is ~46)
SEG = 8                 # reduce segment width; slot widths pad to this
BANK = 512              # fp32 cols per PSUM bank (= max matmul width)
GROUP_BANKS = 2         # banks per exp/reduce group
NEG_BIG = -1e30
KR = 15                 # contraction rows: 5 features x 3 W bands
BF16 = ml_dtypes.bfloat16


def _split3(x):
    a0 = x.astype(BF16)
    r1 = x - a0.astype(np.float64)
    a1 = r1.astype(BF16)
    r2 = r1 - a1.astype(np.float64)
    a2 = r2.astype(BF16)
    return a0, a1, a2


def _prepare(grid_points, X, aw_table, bw_table, elements, C_expand):
    gp = np.asarray(grid_points).astype(np.float64)
    Ng = gp.shape[0]

    mask = (np.asarray(elements) != EXCLUDED_ELEM) & (np.asarray(C_expand) == 1)
    Xa = np.asarray(X).astype(np.float64)[mask]
    el = np.asarray(elements)[mask]
    aw = np.asarray(aw_table).astype(np.float64)[el]
    bw = np.asarray(bw_table).astype(np.float64)[el]
    logaw = np.where(aw > 0, np.log(np.maximum(aw, 1e-300)), NEG_BIG)

    # ---- spatial sort into tiles of 128 points ----
    ntiles = -(-Ng // P)
    ntiles = -(-ntiles // NCORES) * NCORES
    cell = np.floor(gp / np.array([2.0, 2.0, 4.0]))
    order = np.lexsort((cell[:, 2], cell[:, 1], cell[:, 0]))
    npad = ntiles * P - Ng
    order_padded = np.concatenate([order, np.full(npad, order[-1], np.int64)])
    gp_s = gp[order_padded].reshape(ntiles, P, 3)

    lo = gp_s.min(axis=1)
    hi = gp_s.max(axis=1)
    center = (lo + hi) / 2

    # ---- per-tile (atom, gaussian) pair selection, value-based ----
    d = np.maximum(lo[:, None, :] - Xa[None], Xa[None] - hi[:, None, :])
    d2 = (np.maximum(d, 0.0) ** 2).sum(-1)
    incl = logaw[None] + bw[None] * d2[:, :, None] >= np.log(EPS)  # [T,Na,6]
    cnt = incl.reshape(ntiles, -1).sum(1)

    # ---- deal tiles to cores by workload rank ----
    nslots = ntiles // NCORES
    rank = np.argsort(-cnt, kind="stable")
    tilemap = rank.reshape(nslots, NCORES)                # [k, c] -> tile id
    pad_k = np.maximum(cnt[tilemap].max(1), SEG)
    pad_k = -(-pad_k // SEG) * SEG
    off = np.concatenate([[0], np.cumsum(pad_k)])         # slot col offsets
    wtot = int(off[-1])

    # ---- shared G pattern (recentred lattice is identical per tile) ----
    gprime = gp_s - center[:, None, :]
    g5 = np.empty((ntiles, 5, P))
    g5[:, 0] = (gprime ** 2).sum(-1)
    g5[:, 1:4] = np.swapaxes(gprime, 1, 2)
    g5[:, 4] = 1.0
    assert np.all(g5 == g5[0]), "tiles do not share one feature pattern"
    assert np.all(g5[0] == g5[0].astype(BF16).astype(np.float64)), \
        "pattern not exact in bf16"
    g0 = g5[0].astype(BF16)
    G = np.concatenate([g0, g0, g0], axis=0)              # [15, 128]

    # ---- per-core W [15, WTOT] via one vectorized 3-band split ----
    W5 = np.empty((NCORES, 5, wtot))
    W5[:] = np.array([0, 0, 0, 0, NEG_BIG])[None, :, None]
    pair_an = [np.nonzero(incl[t]) for t in range(ntiles)]
    for k in range(nslots):
        q0 = off[k]
        for c in range(NCORES):
            t = int(tilemap[k, c])
            a_i, n_i = pair_an[t]
            m = a_i.shape[0]
            if m == 0:
                continue
            Xp = Xa[a_i] - center[t]
            bwi = bw[a_i, n_i]
            W5[c, 0, q0:q0 + m] = bwi
            W5[c, 1:4, q0:q0 + m] = -2.0 * bwi * Xp.T
            W5[c, 4, q0:q0 + m] = bwi * (Xp ** 2).sum(-1) + logaw[a_i, n_i]
    w0, w1, w2 = _split3(W5)
    Wc = np.concatenate([w0, w1, w2], axis=1)             # [NC, 15, WTOT]

    # ---- banks / groups / DMA chunks ----
    nbanks = -(-wtot // BANK)
    banks = [(b * BANK, min(BANK, wtot - b * BANK)) for b in range(nbanks)]
    if nbanks > 5:
        cb = [1, 3, 5, nbanks]                            # chunk ends (banks)
    elif nbanks > 3:
        cb = [1, 3, nbanks]
    else:
        cb = [nbanks]
    chunks = []                                           # (col0, col1)
    prev = 0
    for e in cb:
        c1 = min(e * BANK, wtot)
        chunks.append((prev, c1))
        prev = c1
    # pair banks into groups; a small trailing bank gets its own group so
    # the final reduce + store cover only a sliver
    gidx = []
    b = 0
    while b < nbanks:
        if b + 1 < nbanks and not (b + 2 == nbanks and banks[-1][1] < 256):
            gidx.append([b, b + 1])
            b += 2
        else:
            gidx.append([b])
            b += 1
    groups = []
    for gb in gidx:
        gbanks = []
        gw = 0
        for bi in gb:
            col0, w = banks[bi]
            ci = next(i for i, (c0, c1) in enumerate(chunks)
                      if c0 <= col0 < c1)
            gbanks.append(dict(loc=gw, w=w, ci=ci, src0=col0 - chunks[ci][0]))
            gw += w
        groups.append(dict(banks=gbanks, gw=gw,
                           seg0=(banks[gb[0]][0]) // SEG))
    meta = dict(
        wtot=wtot, chunks=chunks, groups=groups, off=off, pad_k=pad_k,
        nslots=nslots, tilemap=tilemap, order_padded=order_padded,
        Ng=Ng, ntiles=ntiles,
    )
    return G, Wc, meta


def _build_program(meta):
    nc = bacc.Bacc("TRN2", target_bir_lowering=False, debug=False,
                   num_devices=NCORES)
    wtot, chunks, groups = meta["wtot"], meta["chunks"], meta["groups"]
    nseg = wtot // SEG
    g_d = nc.dram_tensor("gp", [KR, P], mybir.dt.bfloat16,
                         kind="ExternalInput")
    w_d = [nc.dram_tensor(f"w{i}", [KR, c1 - c0], mybir.dt.bfloat16,
                          kind="ExternalInput")
           for i, (c0, c1) in enumerate(chunks)]
    out_d = nc.dram_tensor("out", [P, nseg], mybir.dt.float16,
                           kind="ExternalOutput")

    with tile.TileContext(nc) as tc:
        with (
            tc.tile_pool(name="data", bufs=1) as data,
            tc.tile_pool(name="ps", bufs=4, space="PSUM") as ps,
            tc.tile_pool(name="work", bufs=4) as work,
            tc.tile_pool(name="accp", bufs=4) as accp,
        ):
            g_sb = data.tile([KR, P], mybir.dt.bfloat16)
            nc.scalar.dma_start(g_sb[:], g_d[:])
            w_sb = []
            wq = [nc.sync, nc.gpsimd, nc.sync, nc.gpsimd]
            for i, (c0, c1) in enumerate(chunks):
                wt = data.tile([KR, c1 - c0], mybir.dt.bfloat16, tag=f"w{i}")
                wq[i % len(wq)].dma_start(wt[:], w_d[i][:])
                w_sb.append(wt)
            # dependency-free warm-up: pulls the exp ACT_TABLE_LOAD into
            # the preamble instead of stalling the first real group
            wu = work.tile([P, 2], mybir.dt.float32, tag="wu")
            nc.vector.memset(wu[:], 0.0)
            nc.scalar.activation(out=wu[:], in_=wu[:],
                                 func=mybir.ActivationFunctionType.Exp)

            for grp in groups:
                gw, ns = grp["gw"], grp["gw"] // SEG
                pst = ps.tile([P, GROUP_BANKS * BANK], mybir.dt.float32,
                              tag="ps")
                for bk in grp["banks"]:
                    nc.tensor.matmul(
                        pst[:, bk["loc"]:bk["loc"] + bk["w"]],
                        g_sb[:, :],
                        w_sb[bk["ci"]][:, bk["src0"]:bk["src0"] + bk["w"]],
                        start=True, stop=True,
                    )
                e3 = work.tile([P, GROUP_BANKS * BANK // SEG, SEG],
                               mybir.dt.float16, tag="e")
                nc.scalar.activation(out=e3[:, :ns, :], in_=pst[:, :gw],
                                     func=mybir.ActivationFunctionType.Exp)
                ac = accp.tile([P, GROUP_BANKS * BANK // SEG],
                               mybir.dt.float16, tag="acc")
                with nc.allow_low_precision("bounded fp16 segment sums; "
                                            "host accumulates in fp64"):
                    nc.vector.tensor_reduce(
                        ac[:, :ns], e3[:, :ns, :],
                        axis=mybir.AxisListType.X, op=mybir.AluOpType.add,
                    )
                nc.sync.dma_start(out_d[:, grp["seg0"]:grp["seg0"] + ns],
                                  ac[:, :ns])
    nc.compile()
    return nc


def _assemble(res, meta):
    ntiles, Ng, nslots = meta["ntiles"], meta["Ng"], meta["nslots"]
    tilemap, off = meta["tilemap"], meta["off"]
    starts = (off[:-1] // SEG).astype(np.int64)
    dens_tiles = np.zeros((ntiles, P), np.float32)
    for c in range(NCORES):
        o = res.results[c]["out"].astype(np.float64)      # [P, nseg]
        red = np.add.reduceat(o, starts, axis=1)          # [P, nslots]
        dens_tiles[tilemap[:, c]] = red.T.astype(np.float32)
    dens = np.zeros(Ng, np.float32)
    dens[meta["order_padded"][:Ng]] = dens_tiles.reshape(-1)[:Ng]
    side = round(Ng ** (1 / 3))
    if side ** 3 == Ng:
        return dens.reshape(side, side, side)
    return dens


def _in_maps(G, Wc, meta):
    maps = []
    for c in range(NCORES):
        m = {"gp": np.ascontiguousarray(G)}
        for i, (c0, c1) in enumerate(meta["chunks"]):
            m[f"w{i}"] = np.ascontiguousarray(Wc[c, :, c0:c1])
        maps.append(m)
    return maps


def kernel(grid_points, X, aw_table, bw_table, elements, C_expand):
    G, Wc, meta = _prepare(grid_points, X, aw_table, bw_table,
                           elements, C_expand)
    nc = _build_program(meta)
    res = run_bass_kernel_spmd(nc, _in_maps(G, Wc, meta),
                               list(range(NCORES)))
    return _assemble(res, meta)
